# revision 1
# baseline (speedup 1.0000x reference)
"""Trainium2 Bass kernel for the Haar-mask MLP (histogram_binning).

Key algorithmic fact: every Haar interval edge is a multiple of 2^-10, so the
reference's masks -- and therefore the entire MLP output -- depend only on
u = floor(t * 1024) (1024 possible values, exact in fp32 since *1024 is a
power-of-two scale).  The whole network collapses to a 1024x3 lookup table,
computed once on host from the tiny weights.  The device work is the
memory-bound part: stream t, compute u, gather LUT[u], stream out.

Device plan (pure data parallel, 8 cores, 16384 elements each):
  - DMA t chunk into SBUF [128p x 128] (host pre-permutes so that partition
    16c+p, slot s holds element 2048c + 16s + p -- exactly the wrapped index
    layout the GpSimd gathers want).
  - u = floor(t*1024) on DVE (exact under any convert rounding mode),
    convert+clamp+scale to a 16-bit index.
  - Replicate the LUT per partition; GpSimd gather per chunk; DMA back.

Gather impl variants (GATHER_IMPL):
  ap3: ap_gather, d=3 rows           -- table [1024,3]/partition, out interleaved
  ic3: indirect_copy, inner=3, idx*3 -- same layout, resident HW-assisted op
  ap1: ap_gather, d=1, planar table  -- partition p holds LUT[:, p%16%3]
  ic1: indirect_copy, inner=1, planar
The *3 variants DMA partitions {16c} (rows of 512x3); the *1 variants DMA the
full tile and the host picks rows {16c+f}.
"""

from contextlib import ExitStack

import numpy as np

import concourse.tile as tile
from concourse import bacc, mybir
from concourse.bass_utils import run_bass_kernel_spmd

N_CORES = 8
B, T, F = 16, 8192, 3
N = B * T                    # 131072 total elements
NPC = N // N_CORES           # 16384 per neuron core
P = 128                      # SBUF partitions
S = NPC // P                 # 128 slots per partition
NBINS = 1024
NCHUNK = 4                   # gather/store pipeline chunks
IDXS = NPC // 8 // NCHUNK    # 512 indices per q7-core per chunk

GATHER_IMPL = "ic1"
RUN_KWARGS = {}              # test harness may set {"trace": True}
LAST_RESULTS = None
_CACHE = {}


def _build_lut(W1, b1, W2, b2, W3, b3):
    """MLP output for each of the 1024 half-interval bins, fp32 math."""
    u = np.arange(NBINS)
    acc = np.zeros((NBINS, W1.shape[1]), np.float32)
    for j in range(10):
        k = u >> (10 - j)                       # floor(t * 2^j) for t in bin u
        idx = (1 << j) - 1 + k                  # level-j block offset + k
        sign = np.where((u >> (9 - j)) & 1 == 0, np.float32(1), np.float32(-1))
        acc = acc + sign[:, None] * W1[idx]
    h = np.maximum(acc + b1, np.float32(0))
    h = np.maximum(h @ W2 + b2, np.float32(0))
    return (h @ W3 + b3).astype(np.float32)     # (1024, 3)


def _build_nc(impl):
    planar = impl.endswith("1")
    use_ic = impl.startswith("ic")
    row = NBINS if planar else NBINS * F        # table row elements/partition
    gw = IDXS if planar else IDXS * F           # gather out elements/partition

    nc = bacc.Bacc("TRN2", target_bir_lowering=False, debug=False,
                   enable_asserts=False, num_devices=N_CORES)
    f32 = mybir.dt.float32
    idt = mybir.dt.uint16 if use_ic else mybir.dt.int16
    t_d = nc.dram_tensor("t", [P, S], f32, kind="ExternalInput")
    lut_d = nc.dram_tensor("lut", [P, row], f32, kind="ExternalInput")
    if planar:
        out_d = nc.dram_tensor("out", [NCHUNK, P, IDXS], f32,
                               kind="ExternalOutput")
    else:
        out_d = nc.dram_tensor("out", [8, NCHUNK, IDXS * F], f32,
                               kind="ExternalOutput")

    with tile.TileContext(nc) as tc, ExitStack() as ctx:
        cpool = ctx.enter_context(tc.tile_pool(name="c", bufs=1))
        gpool = ctx.enter_context(tc.tile_pool(name="g", bufs=1))

        t_sb = cpool.tile([P, S], f32)
        nc.sync.dma_start(t_sb[:], t_d[:, :])

        # split the table broadcast across partition quarters AND across
        # engines, so each lands on its own HWDGE queue (the broadcast gates
        # the first gather; same-engine splits would serialize on one queue)
        tab = cpool.tile([P, row], f32)
        for q, eng in enumerate((nc.sync, nc.scalar, nc.sync, nc.scalar)):
            eng.dma_start(tab[q * 32:(q + 1) * 32, :],
                          lut_d[q * 32:(q + 1) * 32, :])

        # exact floor(t*1024): round-to-int (any rounding mode), then
        # subtract 1 wherever the rounded value exceeds the true value
        uf = cpool.tile([P, S], f32)
        ii = cpool.tile([P, S], mybir.dt.int32)
        fb = cpool.tile([P, S], f32)
        adj = cpool.tile([P, S], f32)
        ui = cpool.tile([P, S], f32)
        idx = cpool.tile([P, S], idt)
        nc.vector.tensor_scalar(uf[:], t_sb[:], 1024.0, None,
                                mybir.AluOpType.mult)
        nc.vector.tensor_copy(ii[:], uf[:])
        nc.vector.tensor_copy(fb[:], ii[:])
        nc.vector.tensor_tensor(adj[:], fb[:], uf[:], mybir.AluOpType.is_gt)
        nc.vector.tensor_sub(ui[:], fb[:], adj[:])
        if use_ic and not planar:               # scale idx by 3 for ranges
            mn = cpool.tile([P, S], f32)
            nc.vector.tensor_scalar(mn[:], ui[:], 1023.0, None,
                                    mybir.AluOpType.min)
            nc.vector.tensor_scalar(idx[:], mn[:], 3.0, None,
                                    mybir.AluOpType.mult)
        else:
            nc.vector.tensor_scalar(idx[:], ui[:], 1023.0, None,
                                    mybir.AluOpType.min)

        spc = S // NCHUNK                        # idx columns per chunk
        for k in range(NCHUNK):
            g = gpool.tile([P, gw], f32, tag=f"g{k}")
            idx_k = idx[:, k * spc:(k + 1) * spc]
            if use_ic:
                d = 1 if planar else F
                nc.gpsimd.indirect_copy(
                    g[:].rearrange("p (n d) -> p n d", d=d),
                    tab[:].rearrange("p (n d) -> p n d", d=d),
                    idx_k, i_know_ap_gather_is_preferred=True)
            else:
                nc.gpsimd.ap_gather(g[:], tab[:], idx_k,
                                    channels=P, num_elems=NBINS,
                                    d=1 if planar else F, num_idxs=IDXS)
            if planar:
                nc.sync.dma_start(out_d.ap()[k, :, :], g[:, :])
            else:
                nc.sync.dma_start(out_d.ap()[:, k, :], g[0:P:16, :])
    nc.compile()
    return nc


def _host_inputs(t, lut):
    planar = GATHER_IMPL.endswith("1")
    if planar:
        lut_rep = np.ascontiguousarray(lut.T[np.arange(P) % 16 % 3])
    else:
        lut_rep = np.ascontiguousarray(
            np.broadcast_to(lut.reshape(-1), (P, NBINS * F)))
    tf = np.ascontiguousarray(np.asarray(t, np.float32)).reshape(-1)
    # SBUF partition 16c+p slot s <- element 2048c + 16s + p of the core chunk
    tperm = (tf.reshape(N_CORES, 8, S, 16).transpose(0, 1, 3, 2)
             .reshape(N_CORES, P, S))
    return tperm, lut_rep


def _host_output(raw):
    """Per-core device output -> (NPC, 3)."""
    if GATHER_IMPL.endswith("1"):
        # raw [NCHUNK, 128, IDXS]; feature f of element (c, 512k+i) is at
        # [k, 16c+f, i]
        r = raw.reshape(NCHUNK, 8, 16, IDXS)[:, :, :F, :]   # k c f i
        return np.ascontiguousarray(r.transpose(1, 0, 3, 2)).reshape(NPC, F)
    # raw [8, NCHUNK, IDXS*F]: (c, k, i*3+f) -> element 2048c + 512k + i
    return raw.reshape(NPC, F)


def kernel(t, W1, b1, W2, b2, W3, b3):
    global LAST_RESULTS
    key = ("nc", GATHER_IMPL)
    if key not in _CACHE:
        _CACHE[key] = _build_nc(GATHER_IMPL)
    nc = _CACHE[key]

    lut = _build_lut(np.asarray(W1, np.float32), np.asarray(b1, np.float32),
                     np.asarray(W2, np.float32), np.asarray(b2, np.float32),
                     np.asarray(W3, np.float32), np.asarray(b3, np.float32))
    tperm, lut_rep = _host_inputs(t, lut)
    in_maps = [{"t": np.ascontiguousarray(tperm[m]), "lut": lut_rep}
               for m in range(N_CORES)]

    res = run_bass_kernel_spmd(nc, in_maps, list(range(N_CORES)), **RUN_KWARGS)
    LAST_RESULTS = res
    outs = [_host_output(res.results[m]["out"]) for m in range(N_CORES)]
    return np.concatenate(outs, axis=0).reshape(B, T, F).astype(np.float32)



# revision 3
# speedup vs baseline: 1.0249x; 1.0249x over previous
"""Trainium2 Bass kernel for the Haar-mask MLP (histogram_binning).

Key algorithmic fact: every Haar interval edge is a multiple of 2^-10, so the
reference's masks -- and therefore the entire MLP output -- depend only on
u = floor(t * 1024) (1024 possible values, exact in fp32 since *1024 is a
power-of-two scale).  The whole network collapses to a 1024x3 lookup table,
computed once on host from the tiny weights.  The device work is the
memory-bound part: stream t, compute u, gather LUT[u], stream out.

Device plan (pure data parallel, 8 cores, 16384 elements each):
  - DMA t chunk into SBUF [128p x 128] (host pre-permutes so that partition
    16c+p, slot s holds element 2048c + 16s + p -- exactly the wrapped index
    layout the GpSimd gathers want).
  - u = floor(t*1024) on DVE (exact under any convert rounding mode),
    convert+clamp to a 16-bit index.
  - Planar LUT per partition (partition 16c+f holds LUT[:, f%3]); gather;
    DMA back only the 3 useful partitions per group of 16.

Perf notes (measured on HW):
  - Each BIR-level INDIRECT_COPY costs ~12.9us of hidden Q7 setup (ucode
    load) before its ~1.3us execution -- 4 chunked gathers = ~57us.
  - ap_gather is a Bass-ISA op whose ucode lives in a loadable library;
    an explicit early load_library overlaps the (one) load with the DMA
    prologue and idx compute, then gathers dispatch at ~100ns.

Impl string: "<kind><d>[:c<K>][:nopre]" e.g. "ap1:c1" (default), "ic1:c4"
  kind: ap = ap_gather, ic = indirect_copy (d=1 planar only)
  cK:   K gather chunks (out-DMA overlap vs per-op cost)
  nopre: skip the explicit early library load
"""

from contextlib import ExitStack

import numpy as np

import concourse.tile as tile
from concourse import bacc, library_config, mybir
from concourse.bass_utils import run_bass_kernel_spmd

N_CORES = 8
B, T, F = 16, 8192, 3
N = B * T                    # 131072 total elements
NPC = N // N_CORES           # 16384 per neuron core
P = 128                      # SBUF partitions
S = NPC // P                 # 128 slots per partition
NBINS = 1024
EPC = NPC // 8               # 2048 elements per q7 core

GATHER_IMPL = "ap1:c1"
RUN_KWARGS = {}              # test harness may set {"trace": True}
LAST_RESULTS = None
_CACHE = {}


def _build_lut(W1, b1, W2, b2, W3, b3):
    """MLP output for each of the 1024 half-interval bins, fp32 math."""
    u = np.arange(NBINS)
    acc = np.zeros((NBINS, W1.shape[1]), np.float32)
    for j in range(10):
        k = u >> (10 - j)                       # floor(t * 2^j) for t in bin u
        idx = (1 << j) - 1 + k                  # level-j block offset + k
        sign = np.where((u >> (9 - j)) & 1 == 0, np.float32(1), np.float32(-1))
        acc = acc + sign[:, None] * W1[idx]
    h = np.maximum(acc + b1, np.float32(0))
    h = np.maximum(h @ W2 + b2, np.float32(0))
    return (h @ W3 + b3).astype(np.float32)     # (1024, 3)


def _parse(impl):
    parts = impl.split(":")
    kind = parts[0]
    nchunk = 1
    preload = True
    for p in parts[1:]:
        if p.startswith("c"):
            nchunk = int(p[1:])
        elif p == "nopre":
            preload = False
    return kind, nchunk, preload


def _build_nc(impl):
    kind, nchunk, preload = _parse(impl)
    use_ic = kind.startswith("ic")

    nc = bacc.Bacc("TRN2", target_bir_lowering=False, debug=False,
                   enable_asserts=False, num_devices=N_CORES)
    f32 = mybir.dt.float32
    idt = mybir.dt.uint16 if use_ic else mybir.dt.int16
    t_d = nc.dram_tensor("t", [P, S], f32, kind="ExternalInput")
    lut_d = nc.dram_tensor("lut", [P, NBINS], f32, kind="ExternalInput")
    out_d = nc.dram_tensor("out", [F, 8, EPC], f32, kind="ExternalOutput")

    with tile.TileContext(nc) as tc, ExitStack() as ctx:
        cpool = ctx.enter_context(tc.tile_pool(name="c", bufs=1))
        gpool = ctx.enter_context(tc.tile_pool(name="g", bufs=1))

        if preload and not use_ic:
            # overlap the one-time ap_gather ucode load with the prologue
            nc.gpsimd.load_library(library_config.ap_gather)

        t_sb = cpool.tile([P, S], f32)
        nc.sync.dma_start(t_sb[:], t_d[:, :])

        # split the table broadcast across partition quarters AND across
        # engines, so each lands on its own HWDGE queue
        tab = cpool.tile([P, NBINS], f32)
        for q, eng in enumerate((nc.sync, nc.scalar, nc.sync, nc.scalar)):
            eng.dma_start(tab[q * 32:(q + 1) * 32, :],
                          lut_d[q * 32:(q + 1) * 32, :])

        # exact floor(t*1024): round-to-int (any rounding mode), then
        # subtract 1 wherever the rounded value exceeds the true value
        uf = cpool.tile([P, S], f32)
        ii = cpool.tile([P, S], mybir.dt.int32)
        fb = cpool.tile([P, S], f32)
        adj = cpool.tile([P, S], f32)
        ui = cpool.tile([P, S], f32)
        idx = cpool.tile([P, S], idt)
        nc.vector.tensor_scalar(uf[:], t_sb[:], 1024.0, None,
                                mybir.AluOpType.mult)
        nc.vector.tensor_copy(ii[:], uf[:])
        nc.vector.tensor_copy(fb[:], ii[:])
        nc.vector.tensor_tensor(adj[:], fb[:], uf[:], mybir.AluOpType.is_gt)
        nc.vector.tensor_sub(ui[:], fb[:], adj[:])
        nc.vector.tensor_scalar(idx[:], ui[:], 1023.0, None,
                                mybir.AluOpType.min)

        spc = S // nchunk            # idx columns per chunk
        w = 16 * spc                 # gathered elements per group per chunk
        oeng = (nc.sync, nc.scalar, nc.sync)
        for k in range(nchunk):
            g = gpool.tile([P, w], f32, tag=f"g{k}")
            idx_k = idx[:, k * spc:(k + 1) * spc]
            if use_ic:
                nc.gpsimd.indirect_copy(
                    g[:].rearrange("p (n d) -> p n d", d=1),
                    tab[:].rearrange("p (n d) -> p n d", d=1),
                    idx_k, i_know_ap_gather_is_preferred=True)
            else:
                nc.gpsimd.ap_gather(g[:], tab[:], idx_k,
                                    channels=P, num_elems=NBINS,
                                    d=1, num_idxs=w)
            # only partitions 16c+f (f<3) hold useful data; one 8-partition
            # strided DMA per feature plane, each on its own engine queue
            for f in range(F):
                oeng[f].dma_start(out_d.ap()[f, :, k * w:(k + 1) * w],
                                  g[f:P:16, :])
    nc.compile()
    return nc


def _host_inputs(t, lut):
    lut_rep = np.ascontiguousarray(lut.T[np.arange(P) % 16 % 3])
    tf = np.ascontiguousarray(np.asarray(t, np.float32)).reshape(-1)
    # SBUF partition 16c+p slot s <- element 2048c + 16s + p of the core chunk
    tperm = (tf.reshape(N_CORES, 8, S, 16).transpose(0, 1, 3, 2)
             .reshape(N_CORES, P, S))
    return tperm, lut_rep


def _host_output(raw):
    """Per-core device output [F, 8, EPC] -> (NPC, 3)."""
    return np.ascontiguousarray(raw.transpose(1, 2, 0)).reshape(NPC, F)


def kernel(t, W1, b1, W2, b2, W3, b3):
    global LAST_RESULTS
    key = ("nc", GATHER_IMPL)
    if key not in _CACHE:
        _CACHE[key] = _build_nc(GATHER_IMPL)
    nc = _CACHE[key]

    lut = _build_lut(np.asarray(W1, np.float32), np.asarray(b1, np.float32),
                     np.asarray(W2, np.float32), np.asarray(b2, np.float32),
                     np.asarray(W3, np.float32), np.asarray(b3, np.float32))
    tperm, lut_rep = _host_inputs(t, lut)
    in_maps = [{"t": np.ascontiguousarray(tperm[m]), "lut": lut_rep}
               for m in range(N_CORES)]

    res = run_bass_kernel_spmd(nc, in_maps, list(range(N_CORES)), **RUN_KWARGS)
    LAST_RESULTS = res
    outs = [_host_output(res.results[m]["out"]) for m in range(N_CORES)]
    return np.concatenate(outs, axis=0).reshape(B, T, F).astype(np.float32)


# revision 10
# speedup vs baseline: 1.0465x; 1.0211x over previous
"""Trainium2 Bass kernel for the Haar-mask MLP (histogram_binning).

Key algorithmic fact: every Haar interval edge is a multiple of 2^-10, so the
reference's masks -- and therefore the entire MLP output -- depend only on
u = floor(t * 1024) (1024 possible values, exact in fp32 since *1024 is a
power-of-two scale).  The whole network collapses to a 1024x3 lookup table,
computed once on host from the tiny weights.  The device work is the
memory-bound part: stream t, compute u, gather LUT[u], stream out.

Device plan (pure data parallel, 8 cores, 16384 elements each):
  - DMA t chunk into SBUF [128p x 128] (host pre-permutes so that partition
    16c+p, slot s holds element 2048c + 16s + p -- exactly the wrapped index
    layout the GpSimd gathers want).
  - u = floor(t*1024) on DVE (exact under any convert rounding mode),
    convert+clamp to a 16-bit index.
  - Planar LUT per partition (partition 16c+f holds LUT[:, f%3]); gather;
    DMA back only the 3 useful partitions per group of 16.

Perf notes (measured on HW):
  - Each BIR-level INDIRECT_COPY costs ~12.9us of hidden Q7 setup (ucode
    load) before its ~1.3us execution -- 4 chunked gathers = ~57us.
  - ap_gather is a Bass-ISA op whose ucode lives in a loadable library;
    an explicit early load_library overlaps the (one) load with the DMA
    prologue and idx compute, then gathers dispatch at ~100ns.

Impl string: "<kind><d>[:c<K>][:nopre]" e.g. "ap1:c1" (default), "ic1:c4"
  kind: ap = ap_gather, ic = indirect_copy (d=1 planar only)
  cK:   K gather chunks (out-DMA overlap vs per-op cost)
  nopre: skip the explicit early library load
"""

from contextlib import ExitStack

import numpy as np

import concourse.tile as tile
from concourse import bacc, library_config, mybir
from concourse.bass_utils import run_bass_kernel_spmd

N_CORES = 8
B, T, F = 16, 8192, 3
N = B * T                    # 131072 total elements
NPC = N // N_CORES           # 16384 per neuron core
P = 128                      # SBUF partitions
S = NPC // P                 # 128 slots per partition
NBINS = 1024
EPC = NPC // 8               # 2048 elements per q7 core

GATHER_IMPL = "ap1:c1"
RUN_KWARGS = {}              # test harness may set {"trace": True}
LAST_RESULTS = None
_CACHE = {}


def _build_lut(W1, b1, W2, b2, W3, b3):
    """MLP output for each of the 1024 half-interval bins, fp32 math."""
    u = np.arange(NBINS)
    acc = np.zeros((NBINS, W1.shape[1]), np.float32)
    for j in range(10):
        k = u >> (10 - j)                       # floor(t * 2^j) for t in bin u
        idx = (1 << j) - 1 + k                  # level-j block offset + k
        sign = np.where((u >> (9 - j)) & 1 == 0, np.float32(1), np.float32(-1))
        acc = acc + sign[:, None] * W1[idx]
    h = np.maximum(acc + b1, np.float32(0))
    h = np.maximum(h @ W2 + b2, np.float32(0))
    return (h @ W3 + b3).astype(np.float32)     # (1024, 3)


def _parse(impl):
    parts = impl.split(":")
    kind = parts[0]
    nchunk = 1
    preload = True
    nvalid = EPC
    for p in parts[1:]:
        if p.startswith("c"):
            nchunk = int(p[1:])
        elif p.startswith("v"):
            nvalid = int(p[1:])
        elif p == "nopre":
            preload = False
    return kind, nchunk, preload, nvalid


def _build_nc(impl):
    kind, nchunk, preload, nvalid = _parse(impl)
    use_ic = kind.startswith("i")
    gdt_np = np.dtype("bfloat16") if "b" in kind else np.float32

    nc = bacc.Bacc("TRN2", target_bir_lowering=False, debug=False,
                   enable_asserts=False, num_devices=N_CORES)
    f32 = mybir.dt.float32
    gdt = mybir.dt.bfloat16 if "b" in kind else f32
    idt = mybir.dt.uint16 if use_ic else mybir.dt.int16
    t_d = nc.dram_tensor("t", [P, S], f32, kind="ExternalInput")
    lut_d = nc.dram_tensor("lut", [P, NBINS], gdt, kind="ExternalInput")
    out_d = nc.dram_tensor("out", [F, 8, EPC], gdt, kind="ExternalOutput")

    with tile.TileContext(nc) as tc, ExitStack() as ctx:
        cpool = ctx.enter_context(tc.tile_pool(name="c", bufs=1))
        gpool = ctx.enter_context(tc.tile_pool(name="g", bufs=1))

        if preload and not use_ic:
            # overlap the one-time ap_gather ucode load with the prologue
            nc.gpsimd.load_library(library_config.ap_gather)

        t_sb = cpool.tile([P, S], f32)
        nc.sync.dma_start(t_sb[:], t_d[:, :])

        # split the table broadcast across partition quarters AND across
        # engines, so each lands on its own HWDGE queue
        tab = cpool.tile([P, NBINS], gdt)
        for q, eng in enumerate((nc.sync, nc.scalar, nc.sync, nc.scalar)):
            eng.dma_start(tab[q * 32:(q + 1) * 32, :],
                          lut_d[q * 32:(q + 1) * 32, :])

        spc = S // nchunk            # idx columns per chunk
        w = 16 * spc                 # gathered elements per group per chunk
        oeng = (nc.sync, nc.scalar, nc.sync)
        for k in range(nchunk):
            nv = nvalid if nchunk == 1 else w
            t_k = t_sb[:, k * spc:(k + 1) * spc]

            # exact floor(t*1024) in 4 DVE ops per chunk (separate tiles per
            # chunk so gather k only waits on its own chunk's index compute):
            # ii = int(t*1024) (any rounding mode), fb = float(ii),
            # adj = (fb*2^-10 > t) i.e. rounded-up, idx = fb-adj.
            # Result is always in [0, 1023] for t in [0,1), so no clamp.
            ii = cpool.tile([P, spc], mybir.dt.int32, tag=f"ii{k}")
            fb = cpool.tile([P, spc], f32, tag=f"fb{k}")
            adj = cpool.tile([P, spc], f32, tag=f"adj{k}")
            idx = cpool.tile([P, spc], idt, tag=f"idx{k}")
            nc.vector.tensor_scalar(ii[:], t_k, 1024.0, None,
                                    mybir.AluOpType.mult)
            nc.vector.tensor_copy(fb[:], ii[:])
            nc.vector.scalar_tensor_tensor(adj[:], fb[:], 2.0 ** -10, t_k,
                                           mybir.AluOpType.mult,
                                           mybir.AluOpType.is_gt)
            nc.vector.scalar_tensor_tensor(idx[:], fb[:], 1.0, adj[:],
                                           mybir.AluOpType.mult,
                                           mybir.AluOpType.subtract)

            g = gpool.tile([P, w], gdt, tag=f"g{k}")
            if use_ic:
                nc.gpsimd.indirect_copy(
                    g[:, :nv].rearrange("p (n d) -> p n d", d=1),
                    tab[:].rearrange("p (n d) -> p n d", d=1),
                    idx[:], i_know_ap_gather_is_preferred=True)
            else:
                nc.gpsimd.ap_gather(g[:, :nv], tab[:], idx[:],
                                    channels=P, num_elems=NBINS,
                                    d=1, num_idxs=nv)
            # only partitions 16c+f (f<3) hold useful data; one 8-partition
            # strided DMA per feature plane, each on its own engine queue
            for f in range(F):
                oeng[f].dma_start(out_d.ap()[f, :, k * w:k * w + nv],
                                  g[f:P:16, :nv])
    nc.compile()
    return nc


def _host_inputs(t, lut, gdt_np=np.float32):
    lut_rep = np.ascontiguousarray(lut.T[np.arange(P) % 16 % 3].astype(gdt_np))
    tf = np.ascontiguousarray(np.asarray(t, np.float32)).reshape(-1)
    # SBUF partition 16c+p slot s <- element 2048c + 16s + p of the core chunk
    tperm = (tf.reshape(N_CORES, 8, S, 16).transpose(0, 1, 3, 2)
             .reshape(N_CORES, P, S))
    return tperm, lut_rep


def _host_output(raw):
    """Per-core device output [F, 8, EPC] -> (NPC, 3)."""
    return np.ascontiguousarray(
        raw.transpose(1, 2, 0).astype(np.float32)).reshape(NPC, F)


def kernel(t, W1, b1, W2, b2, W3, b3):
    global LAST_RESULTS
    key = ("nc", GATHER_IMPL)
    if key not in _CACHE:
        _CACHE[key] = _build_nc(GATHER_IMPL)
    nc = _CACHE[key]

    lut = _build_lut(np.asarray(W1, np.float32), np.asarray(b1, np.float32),
                     np.asarray(W2, np.float32), np.asarray(b2, np.float32),
                     np.asarray(W3, np.float32), np.asarray(b3, np.float32))
    kind = _parse(GATHER_IMPL)[0]
    gdt_np = np.dtype("bfloat16") if "b" in kind else np.float32
    tperm, lut_rep = _host_inputs(t, lut, gdt_np)
    in_maps = [{"t": np.ascontiguousarray(tperm[m]), "lut": lut_rep}
               for m in range(N_CORES)]

    res = run_bass_kernel_spmd(nc, in_maps, list(range(N_CORES)), **RUN_KWARGS)
    LAST_RESULTS = res
    outs = [_host_output(res.results[m]["out"]) for m in range(N_CORES)]
    full = np.concatenate(outs, axis=0)

    # elements j >= nvalid of each 2048-element group are not gathered on
    # device (ISA dst-elem-count limit); fill them from the same LUT here
    _, nchunk, _, nvalid = _parse(GATHER_IMPL)
    if nchunk == 1 and nvalid < EPC:
        tf = np.ascontiguousarray(np.asarray(t, np.float32)).reshape(-1)
        pos = (np.arange(N).reshape(N_CORES, 8, EPC)[:, :, nvalid:]).reshape(-1)
        u = np.floor(tf[pos] * np.float32(1024.0)).astype(np.int64)
        full[pos] = lut[np.clip(u, 0, NBINS - 1)]

    return full.reshape(B, T, F).astype(np.float32)


# revision 11
# speedup vs baseline: 1.0522x; 1.0054x over previous
"""Trainium2 Bass kernel for the Haar-mask MLP (histogram_binning).

Key algorithmic fact: every Haar interval edge is a multiple of 2^-10, so the
reference's masks -- and therefore the entire MLP output -- depend only on
u = floor(t * 1024) (1024 possible values, exact in fp32 since *1024 is a
power-of-two scale).  The whole network collapses to a 1024x3 lookup table,
computed once on host from the tiny weights.  The device work is the
memory-bound part: stream t, compute u, gather LUT[u], stream out.

Device plan (pure data parallel, 8 cores, 16384 elements each):
  - DMA t chunk into SBUF [128p x 128] (host pre-permutes so that partition
    16c+p, slot s holds element 2048c + 16s + p -- exactly the wrapped index
    layout the GpSimd gathers want).
  - u = floor(t*1024) on DVE (exact under any convert rounding mode),
    convert+clamp to a 16-bit index.
  - Planar LUT per partition (partition 16c+f holds LUT[:, f%3]); gather;
    DMA back only the 3 useful partitions per group of 16.

Perf notes (measured on HW):
  - Each BIR-level INDIRECT_COPY costs ~12.9us of hidden Q7 setup (ucode
    load) before its ~1.3us execution -- 4 chunked gathers = ~57us.
  - ap_gather is a Bass-ISA op whose ucode lives in a loadable library;
    an explicit early load_library overlaps the (one) load with the DMA
    prologue and idx compute, then gathers dispatch at ~100ns.

Impl string: "<kind><d>[:c<K>][:nopre]" e.g. "ap1:c1" (default), "ic1:c4"
  kind: ap = ap_gather, ic = indirect_copy (d=1 planar only)
  cK:   K gather chunks (out-DMA overlap vs per-op cost)
  nopre: skip the explicit early library load
"""

from contextlib import ExitStack

import numpy as np

import concourse.tile as tile
from concourse import bacc, library_config, mybir
from concourse.bass_utils import run_bass_kernel_spmd

N_CORES = 8
B, T, F = 16, 8192, 3
N = B * T                    # 131072 total elements
NPC = N // N_CORES           # 16384 per neuron core
P = 128                      # SBUF partitions
S = NPC // P                 # 128 slots per partition
NBINS = 1024
EPC = NPC // 8               # 2048 elements per q7 core

GATHER_IMPL = "ap1:c1"
RUN_KWARGS = {}              # test harness may set {"trace": True}
LAST_RESULTS = None
_CACHE = {}


def _build_lut(W1, b1, W2, b2, W3, b3):
    """MLP output for each of the 1024 half-interval bins, fp32 math."""
    u = np.arange(NBINS)
    acc = np.zeros((NBINS, W1.shape[1]), np.float32)
    for j in range(10):
        k = u >> (10 - j)                       # floor(t * 2^j) for t in bin u
        idx = (1 << j) - 1 + k                  # level-j block offset + k
        sign = np.where((u >> (9 - j)) & 1 == 0, np.float32(1), np.float32(-1))
        acc = acc + sign[:, None] * W1[idx]
    h = np.maximum(acc + b1, np.float32(0))
    h = np.maximum(h @ W2 + b2, np.float32(0))
    return (h @ W3 + b3).astype(np.float32)     # (1024, 3)


def _parse(impl):
    parts = impl.split(":")
    kind = parts[0]
    nchunk = 1
    preload = True
    nvalid = EPC
    for p in parts[1:]:
        if p.startswith("c"):
            nchunk = int(p[1:])
        elif p.startswith("v"):
            nvalid = int(p[1:])
        elif p == "nopre":
            preload = False
    return kind, nchunk, preload, nvalid


def _build_nc(impl):
    kind, nchunk, preload, nvalid = _parse(impl)
    use_ic = kind.startswith("i")
    gdt_np = np.dtype("bfloat16") if "b" in kind else np.float32

    nc = bacc.Bacc("TRN2", target_bir_lowering=False, debug=False,
                   enable_asserts=False, num_devices=N_CORES)
    f32 = mybir.dt.float32
    gdt = mybir.dt.bfloat16 if "b" in kind else f32
    idt = mybir.dt.uint16 if use_ic else mybir.dt.int16
    t_d = nc.dram_tensor("t", [P, S], f32, kind="ExternalInput")
    lut_d = nc.dram_tensor("lut", [P, NBINS], gdt, kind="ExternalInput")
    out_d = nc.dram_tensor("out", [F, 8, EPC], gdt, kind="ExternalOutput")

    with tile.TileContext(nc) as tc, ExitStack() as ctx:
        cpool = ctx.enter_context(tc.tile_pool(name="c", bufs=1))
        gpool = ctx.enter_context(tc.tile_pool(name="g", bufs=1))

        if preload and not use_ic:
            # overlap the one-time ap_gather ucode load with the prologue
            nc.gpsimd.load_library(library_config.ap_gather)

        # t alone on the sync queue (gates the DVE index chain), the whole
        # table as ONE dma on scalar (HWDGE issue cost dominates transfer
        # for these sizes -- one 256KB bf16 dma beats four quarter dmas)
        t_sb = cpool.tile([P, S], f32)
        nc.sync.dma_start(t_sb[:], t_d[:, :])
        tab = cpool.tile([P, NBINS], gdt)
        nc.scalar.dma_start(tab[:], lut_d[:, :])

        spc = S // nchunk            # idx columns per chunk
        w = 16 * spc                 # gathered elements per group per chunk
        oeng = (nc.sync, nc.scalar, nc.sync)
        for k in range(nchunk):
            nv = nvalid if nchunk == 1 else w
            t_k = t_sb[:, k * spc:(k + 1) * spc]

            # exact floor(t*1024) in 4 DVE ops per chunk (separate tiles per
            # chunk so gather k only waits on its own chunk's index compute):
            # ii = int(t*1024) (any rounding mode), fb = float(ii),
            # adj = (fb*2^-10 > t) i.e. rounded-up, idx = fb-adj.
            # Result is always in [0, 1023] for t in [0,1), so no clamp.
            ii = cpool.tile([P, spc], mybir.dt.int32, tag=f"ii{k}")
            fb = cpool.tile([P, spc], f32, tag=f"fb{k}")
            adj = cpool.tile([P, spc], f32, tag=f"adj{k}")
            idx = cpool.tile([P, spc], idt, tag=f"idx{k}")
            nc.vector.tensor_scalar(ii[:], t_k, 1024.0, None,
                                    mybir.AluOpType.mult)
            nc.vector.tensor_copy(fb[:], ii[:])
            nc.vector.scalar_tensor_tensor(adj[:], fb[:], 2.0 ** -10, t_k,
                                           mybir.AluOpType.mult,
                                           mybir.AluOpType.is_gt)
            nc.vector.scalar_tensor_tensor(idx[:], fb[:], 1.0, adj[:],
                                           mybir.AluOpType.mult,
                                           mybir.AluOpType.subtract)

            g = gpool.tile([P, w], gdt, tag=f"g{k}")
            if use_ic:
                nc.gpsimd.indirect_copy(
                    g[:, :nv].rearrange("p (n d) -> p n d", d=1),
                    tab[:].rearrange("p (n d) -> p n d", d=1),
                    idx[:], i_know_ap_gather_is_preferred=True)
            else:
                nc.gpsimd.ap_gather(g[:, :nv], tab[:], idx[:],
                                    channels=P, num_elems=NBINS,
                                    d=1, num_idxs=nv)
            # only partitions 16c+f (f<3) hold useful data; one 8-partition
            # strided DMA per feature plane, each on its own engine queue
            for f in range(F):
                oeng[f].dma_start(out_d.ap()[f, :, k * w:k * w + nv],
                                  g[f:P:16, :nv])
    nc.compile()
    return nc


def _host_inputs(t, lut, gdt_np=np.float32):
    lut_rep = np.ascontiguousarray(lut.T[np.arange(P) % 16 % 3].astype(gdt_np))
    tf = np.ascontiguousarray(np.asarray(t, np.float32)).reshape(-1)
    # SBUF partition 16c+p slot s <- element 2048c + 16s + p of the core chunk
    tperm = (tf.reshape(N_CORES, 8, S, 16).transpose(0, 1, 3, 2)
             .reshape(N_CORES, P, S))
    return tperm, lut_rep


def _host_output(raw):
    """Per-core device output [F, 8, EPC] -> (NPC, 3)."""
    return np.ascontiguousarray(
        raw.transpose(1, 2, 0).astype(np.float32)).reshape(NPC, F)


def kernel(t, W1, b1, W2, b2, W3, b3):
    global LAST_RESULTS
    key = ("nc", GATHER_IMPL)
    if key not in _CACHE:
        _CACHE[key] = _build_nc(GATHER_IMPL)
    nc = _CACHE[key]

    lut = _build_lut(np.asarray(W1, np.float32), np.asarray(b1, np.float32),
                     np.asarray(W2, np.float32), np.asarray(b2, np.float32),
                     np.asarray(W3, np.float32), np.asarray(b3, np.float32))
    kind = _parse(GATHER_IMPL)[0]
    gdt_np = np.dtype("bfloat16") if "b" in kind else np.float32
    tperm, lut_rep = _host_inputs(t, lut, gdt_np)
    in_maps = [{"t": np.ascontiguousarray(tperm[m]), "lut": lut_rep}
               for m in range(N_CORES)]

    res = run_bass_kernel_spmd(nc, in_maps, list(range(N_CORES)), **RUN_KWARGS)
    LAST_RESULTS = res
    outs = [_host_output(res.results[m]["out"]) for m in range(N_CORES)]
    full = np.concatenate(outs, axis=0)

    # elements j >= nvalid of each 2048-element group are not gathered on
    # device (ISA dst-elem-count limit); fill them from the same LUT here
    _, nchunk, _, nvalid = _parse(GATHER_IMPL)
    if nchunk == 1 and nvalid < EPC:
        tf = np.ascontiguousarray(np.asarray(t, np.float32)).reshape(-1)
        pos = (np.arange(N).reshape(N_CORES, 8, EPC)[:, :, nvalid:]).reshape(-1)
        u = np.floor(tf[pos] * np.float32(1024.0)).astype(np.int64)
        full[pos] = lut[np.clip(u, 0, NBINS - 1)]

    return full.reshape(B, T, F).astype(np.float32)
